# revision 25
# baseline (speedup 1.0000x reference)
"""Trainium2 Bass kernel for the GaussianModel occupancy-grid problem.

Strategy
--------
occ[p] = sum_g w(g, block) * exp(power(p, g)) where power is a quadratic
form in the voxel coordinate p.  We rewrite power as a rank-11 inner
product  Phi(p) . c_g  (6 quadratic + 3 linear + 2 per-block const rows
selected by indicator features) and fold the per-(gaussian, block) weight
w into the constant coefficient as log(w) (w == 0 -> -1e10, which
underflows exp to exactly 0).

Only ~1.5% of (gaussian, block) pairs pass the reference's box test; the
host compacts, per pair of z-adjacent blocks (128 voxels = full partition
dim), the union of active gaussians into bf16 coefficient columns.

Device pipeline per core (SPMD on 8 cores):
    PE:   power = lhsT^T @ rhs -> PSUM f32.  Work is packed densely into
          512-col PSUM banks (items split at bank boundaries), and up to
          11 items are stacked block-diagonally per matmul (11 feature
          rows each) so the instruction count stays low.
    ACT:  one exp over each 4-bank (2048 col) PSUM batch -> SBUF f32
    DVE:  segmented tensor_reduce over gaussians -> val columns
    Pool: halves the biggest chains in place (tensor_tensor add) before
          DVE finishes them, balancing the per-batch drain.

All inputs live in one [128, W] bf16 "wall": per stacked group, an
[11n, 128] lhsT block then an [11n, glen] rhs block, so each batch's data
arrives as a single tight rectangle DMA (row count = max stack depth of
that span, which grows with batch index as items shrink).

A common descending length schedule (elementwise max across cores) keeps
shapes identical across cores.  Coordinates are re-centered per pair to
avoid catastrophic cancellation in the expanded quadratic (bf16 inputs,
f32 PSUM accumulate; rel err ~8e-4 vs the 2e-2 budget).
"""

import numpy as np

NB = 16          # num_blocks
RES = 64         # resolution
SPLIT = 4        # voxels per block side
N_CORES = 8
K_FEAT = 11      # 9 shared features + 2 block-indicator/const rows
MAX_CHUNK = 512  # max gaussians per work item
BANK = 512       # PSUM bank, f32 cols per partition
BANKS_PER_BATCH = 4
BATCH_COLS = BANK * BANKS_PER_BATCH
STACK_MAX = 11   # items per stacked matmul (11*11 = 121 <= 128)

NEG = np.float32(-1e10)

_CACHE = {}


def _host_prep(_xyz, _scaling, _rotation, _opacity):
    """Mirror of the reference's per-gaussian preprocessing (numpy fp32)."""
    f32 = np.float32
    opac = (1.0 / (1.0 + np.exp(-_opacity[:, 0].astype(f32)))).astype(f32)
    keep = opac > 0.005
    opa = np.where(keep, opac, f32(0.0)).astype(f32)

    BIG = f32(1e10)
    mn = np.min(np.where(keep[:, None], _xyz, BIG), axis=0)
    mx = np.max(np.where(keep[:, None], _xyz, -BIG), axis=0)
    center = ((mn + mx) / 2).astype(f32)
    scale = (f32(1.8) / np.max(mx - mn)).astype(f32)
    xyzs = ((_xyz - center) * scale).astype(f32)
    stds = (np.exp(_scaling) * scale).astype(f32)

    q = (_rotation / np.linalg.norm(_rotation, axis=1, keepdims=True)).astype(f32)
    r, x, y, z = q[:, 0], q[:, 1], q[:, 2], q[:, 3]
    R = np.stack([
        np.stack([1 - 2 * (y * y + z * z), 2 * (x * y - r * z), 2 * (x * z + r * y)], -1),
        np.stack([2 * (x * y + r * z), 1 - 2 * (x * x + z * z), 2 * (y * z - r * x)], -1),
        np.stack([2 * (x * z - r * y), 2 * (y * z + r * x), 1 - 2 * (x * x + y * y)], -1),
    ], axis=1).astype(f32)
    L = R * stds[:, None, :]
    C = np.einsum('nij,nkj->nik', L, L).astype(f32)
    a, b, c = C[:, 0, 0], C[:, 0, 1], C[:, 0, 2]
    d, e, f = C[:, 1, 1], C[:, 1, 2], C[:, 2, 2]
    inv_det = (1.0 / (a * d * f + 2 * e * c * b - e * e * a - c * c * d
                      - b * b * f + 1e-24)).astype(f32)
    ia = ((d * f - e * e) * inv_det).astype(f32)
    ib = ((e * c - b * f) * inv_det).astype(f32)
    ic = ((e * b - c * d) * inv_det).astype(f32)
    id_ = ((a * f - c * c) * inv_det).astype(f32)
    ie = ((b * c - e * a) * inv_det).astype(f32)
    if_ = ((a * d - b * b) * inv_det).astype(f32)

    logopa = np.where(opa > 0, np.log(np.maximum(opa, 1e-30)),
                      NEG).astype(f32)
    return xyzs, opa, logopa, (ia, ib, ic, id_, ie, if_)


def _build_workload(xyzs, opa, logopa, inv):
    """Enumerate per-pair active unions, deal to cores, pack densely.

    Returns (schedule, per_core) where schedule is identical across cores.
    """
    f32 = np.float32
    lin = np.linspace(-1.0, 1.0, RES).astype(f32)
    relax = f32((2.0 / NB) * 1.5)
    gx, gy, gz = xyzs[:, 0], xyzs[:, 1], xyzs[:, 2]
    act = opa > 0

    vmin = lin[np.arange(NB) * SPLIT] - relax
    vmax = lin[np.arange(NB) * SPLIT + SPLIT - 1] + relax
    Fx = (gx[None, :] > vmin[:, None]) & (gx[None, :] < vmax[:, None])
    Fy = (gy[None, :] > vmin[:, None]) & (gy[None, :] < vmax[:, None])
    Fz = (gz[None, :] > vmin[:, None]) & (gz[None, :] < vmax[:, None])
    Fz_pair = (Fz & act).reshape(NB // 2, 2, -1)

    chunks = []  # (length, bi, bj, m, gauss-index-array)
    for bi in range(NB):
        fx = Fx[bi]
        for bj in range(NB):
            fxy = fx & Fy[bj] & act
            if not fxy.any():
                continue
            for m in range(NB // 2):
                un = fxy & (Fz_pair[m, 0] | Fz_pair[m, 1])
                idx = np.nonzero(un)[0]
                for s in range(0, idx.size, MAX_CHUNK):
                    part = idx[s:s + MAX_CHUNK]
                    chunks.append((part.size, bi, bj, m, part))
    chunks.sort(key=lambda t: -t[0])

    m_items = (len(chunks) + N_CORES - 1) // N_CORES
    # deal round-robin: chunk k -> core k % 8, rank k // 8.  Schedule length
    # per rank = max over cores (the first core's, lengths descending),
    # rounded up to a multiple of 4 for DMA alignment.
    l_sched = []
    for rank in range(m_items):
        lmax = chunks[rank * N_CORES][0]
        l_sched.append(max(4, (lmax + 3) // 4 * 4))

    per_core = [[None] * m_items for _ in range(N_CORES)]
    for k, ch in enumerate(chunks):
        per_core[k % N_CORES][k // N_CORES] = ch

    cum = np.concatenate([[0], np.cumsum(l_sched)]).astype(np.int64)
    tot = int(cum[-1])

    # --- dense packing: split items at bank boundaries -------------------
    frags = []  # dicts: rank, fofs (offset within item), flen, g0 (global col)
    for rank in range(m_items):
        o = int(cum[rank])
        left = l_sched[rank]
        fofs = 0
        while left > 0:
            room = BANK - (o % BANK)
            t = min(room, left)
            frags.append({"rank": rank, "fofs": fofs, "flen": t, "g0": o})
            o += t
            left -= t
            fofs += t
    n_frags = len(frags)

    # --- stacked matmul groups (per bank, <= STACK_MAX frags each) -------
    groups = []  # dicts: g0, glen, members, rows, lhs_ofs, rhs_ofs
    cur = None
    for fi, f in enumerate(frags):
        bank = f["g0"] // BANK
        if cur is None or bank != cur["bank"] or len(cur["members"]) == STACK_MAX:
            cur = {"bank": bank, "g0": f["g0"], "glen": 0, "members": []}
            groups.append(cur)
        cur["members"].append(fi)
        cur["glen"] += f["flen"]
    wall_w = 0
    for g in groups:
        g["rows"] = K_FEAT * len(g["members"])
        g["lhs_ofs"] = wall_w
        g["rhs_ofs"] = wall_w + 128
        wall_w += 128 + g["glen"]
        del g["bank"]

    n_batches = (tot + BATCH_COLS - 1) // BATCH_COLS

    # --- batch emission order: first batches of 512-col items both lead
    # (tiny 11-row DMA rectangles) and, moved to the end, drain fastest
    # (single foldable chains), so put the last two 512-batches at the
    # tail and start from batch 2.
    if n_batches > 4:
        batch_order = list(range(2, n_batches)) + [0, 1]
    else:
        batch_order = list(range(n_batches))

    # --- per-batch wall rectangles + progressive DMA plan ----------------
    wb_lo = [None] * n_batches
    wb_hi = [None] * n_batches
    wb_rows = [0] * n_batches
    for g in groups:
        b = g["g0"] // BATCH_COLS
        if wb_lo[b] is None:
            wb_lo[b] = g["lhs_ofs"]
        wb_hi[b] = g["lhs_ofs"] + 128 + g["glen"]
        wb_rows[b] = max(wb_rows[b], g["rows"])
    # one DMA rectangle per batch in emission order; merge runs of equal
    # row count (max 2 batches per DMA) to bound the DMA count
    dma_plan = []  # (rows, col_lo, col_hi) -- col ranges are contiguous
    i = 0
    while i < len(batch_order):
        b = batch_order[i]
        if (i + 1 < len(batch_order) and batch_order[i + 1] == b + 1
                and wb_rows[b + 1] == wb_rows[b] and i > 0):
            dma_plan.append((wb_rows[b], wb_lo[b], wb_hi[b + 1]))
            i += 2
        else:
            dma_plan.append((wb_rows[b], wb_lo[b], wb_hi[b]))
            i += 1

    # --- val slots in batch-emission order (final val DMA = one suffix) --
    order_pos = {b: i for i, b in enumerate(batch_order)}
    for si, f in enumerate(sorted(
            range(n_frags),
            key=lambda i: (order_pos[frags[i]["g0"] // BATCH_COLS],
                           frags[i]["g0"]))):
        frags[f]["slot"] = si

    # --- reduce chains: runs of same-flen frags within a batch -----------
    chains = []  # dicts: slot0, n, flen, batch, ofs (col offset in batch)
    i = 0
    while i < n_frags:
        f = frags[i]
        b0 = f["g0"] // BATCH_COLS
        L = f["flen"]
        j = i
        while (j + 1 < n_frags and frags[j + 1]["flen"] == L
               and frags[j + 1]["g0"] // BATCH_COLS == b0):
            j += 1
        chains.append({"slot0": frags[i]["slot"], "n": j - i + 1, "flen": L,
                       "batch": b0, "ofs": f["g0"] - b0 * BATCH_COLS})
        i = j + 1

    # --- engine assignment: DVE reduce vs Pool folds ---------------------
    # Pool (gpsimd) halves a sub-chain in place with tensor_tensor adds
    # (~2 ns/elem + overhead); DVE then reduces the remaining quarter or
    # half.  Chains are splittable at item granularity, so balance
    # globally: straight-DVE items vs Pool-folded items (~DVE 1.04 vs
    # Pool 1.98 ns/elem => Pool takes roughly a third of the columns).
    # the last emitted batch's items fuse exp+reduce on ACT via accum_out
    # (ACT is idle at the tail; kills the act->fold->reduce serial chain)
    accum_batch = batch_order[-1]
    if sum(1 for f in frags if f["g0"] // BATCH_COLS == accum_batch) > 6:
        accum_batch = -1

    t_d, t_p = 0.0, 0.0
    for ch in sorted(chains, key=lambda c: -c["n"] * c["flen"]):
        if ch["batch"] == accum_batch:
            ch["n_p"] = 0
            ch["folds"] = []
            ch["l_fin"] = ch["flen"]
            ch["accum"] = True
            continue
        ch["accum"] = False
        n, L = ch["n"], ch["flen"]
        best = None
        for n_p in range(0, n + 1):   # items whose folds go to Pool
            for k in ((1, 2, 3) if n_p else (0,)):
                folds = []
                L0 = L
                ok = True
                for _ in range(k):
                    half = L0 // 2
                    if n_p * half < 64:
                        ok = False
                        break
                    folds.append((L0, L0 - half, half))
                    L0 = L0 - half
                if not ok:
                    continue
                c_pool = sum(1.984 * n_p * fo[2] + 160.0 for fo in folds)
                c_dve = 0.0
                if n - n_p:
                    c_dve += 1.17 * (n - n_p) * L + 140.0
                if n_p:
                    c_dve += 1.17 * n_p * L0 + 140.0
                mk = max(t_d + c_dve, t_p + c_pool)
                if best is None or mk < best[0]:
                    best = (mk, n_p, folds, L0, c_pool, c_dve)
        _, n_p, folds, L0, c_pool, c_dve = best
        ch["n_p"] = n_p         # last n_p items are pool-folded
        ch["folds"] = folds
        ch["l_fin"] = L0
        t_p += c_pool
        t_d += c_dve

    schedule = {
        "m_items": m_items,
        "l_sched": l_sched,
        "cum": cum,
        "tot": tot,
        "frags": frags,
        "groups": groups,
        "wall_w": wall_w,
        "dma_plan": dma_plan,
        "batch_order": batch_order,
        "accum_batch": accum_batch,
        "n_batches": n_batches,
        "chains": chains,
    }
    return schedule, per_core


def _build_inputs(schedule, per_core, xyzs, logopa, inv):
    """Build per-core [128, W] bf16 wall arrays."""
    import ml_dtypes
    bf16 = ml_dtypes.bfloat16
    f32 = np.float32
    ia, ib, ic, id_, ie, if_ = inv
    lin = np.linspace(-1.0, 1.0, RES).astype(f32)
    relax = f32((2.0 / NB) * 1.5)
    gx, gy, gz = xyzs[:, 0], xyzs[:, 1], xyzs[:, 2]

    m_items = schedule["m_items"]
    l_sched = schedule["l_sched"]
    frags = schedule["frags"]
    groups = schedule["groups"]
    wall_w = schedule["wall_w"]

    in_maps = []
    for c in range(N_CORES):
        lhs_rank = np.zeros((m_items, K_FEAT, 128), f32)
        rhs_rank = [None] * m_items
        for rank in range(m_items):
            ch = per_core[c][rank]
            Lr = l_sched[rank]
            R = np.zeros((K_FEAT, Lr), f32)
            R[9:11, :] = NEG
            if ch is None:
                lhs_rank[rank, 9, :64] = 1.0
                lhs_rank[rank, 10, 64:] = 1.0
                rhs_rank[rank] = R
                continue
            _, bi, bj, m, idx = ch
            xs = lin[bi * 4:bi * 4 + 4]
            ys = lin[bj * 4:bj * 4 + 4]
            zs = lin[8 * m:8 * m + 8]
            cx = f32((xs[0] + xs[3]) * 0.5)
            cy = f32((ys[0] + ys[3]) * 0.5)
            cz = f32((zs[0] + zs[7]) * 0.5)
            xl, yl, zl = xs - cx, ys - cy, zs - cz

            X, Y, Z = np.meshgrid(xl, yl, zl[:4], indexing='ij')
            P0 = np.stack([X.ravel(), Y.ravel(), Z.ravel()], 0)
            X, Y, Z = np.meshgrid(xl, yl, zl[4:], indexing='ij')
            P1 = np.stack([X.ravel(), Y.ravel(), Z.ravel()], 0)
            P = np.concatenate([P0, P1], 1).astype(f32)  # [3, 128]
            px, py, pz = P[0], P[1], P[2]
            lhs_rank[rank, 0] = px * px
            lhs_rank[rank, 1] = py * py
            lhs_rank[rank, 2] = pz * pz
            lhs_rank[rank, 3] = px * py
            lhs_rank[rank, 4] = px * pz
            lhs_rank[rank, 5] = py * pz
            lhs_rank[rank, 6] = px
            lhs_rank[rank, 7] = py
            lhs_rank[rank, 8] = pz
            lhs_rank[rank, 9, :64] = 1.0
            lhs_rank[rank, 10, 64:] = 1.0

            g0x = (gx[idx] - cx).astype(f32)
            g0y = (gy[idx] - cy).astype(f32)
            g0z = (gz[idx] - cz).astype(f32)
            A_ = ia[idx]; B_ = ib[idx]; Cc = ic[idx]
            D_ = id_[idx]; E_ = ie[idx]; F_ = if_[idx]
            Agx = A_ * g0x + B_ * g0y + Cc * g0z
            Agy = B_ * g0x + D_ * g0y + E_ * g0z
            Agz = Cc * g0x + E_ * g0y + F_ * g0z
            const = (-0.5 * (g0x * Agx + g0y * Agy + g0z * Agz)).astype(f32)

            n = idx.size
            R[0, :n] = -0.5 * A_
            R[1, :n] = -0.5 * D_
            R[2, :n] = -0.5 * F_
            R[3, :n] = -B_
            R[4, :n] = -Cc
            R[5, :n] = -E_
            R[6, :n] = Agx
            R[7, :n] = Agy
            R[8, :n] = Agz
            in_b0 = ((gz[idx] > lin[8 * m] - relax)
                     & (gz[idx] < lin[8 * m + 3] + relax))
            in_b1 = ((gz[idx] > lin[8 * m + 4] - relax)
                     & (gz[idx] < lin[8 * m + 7] + relax))
            base = const + logopa[idx]
            R[9, :n] = np.where(in_b0, base, NEG)
            R[10, :n] = np.where(in_b1, base, NEG)
            rhs_rank[rank] = R

        wall = np.zeros((128, wall_w), f32)
        for g in groups:
            col = g["rhs_ofs"]
            for j, fi in enumerate(g["members"]):
                f = frags[fi]
                rank = f["rank"]
                r0 = K_FEAT * j
                wall[r0:r0 + K_FEAT, g["lhs_ofs"]:g["lhs_ofs"] + 128] = \
                    lhs_rank[rank]
                wall[r0:r0 + K_FEAT, col:col + f["flen"]] = \
                    rhs_rank[rank][:, f["fofs"]:f["fofs"] + f["flen"]]
                col += f["flen"]
        in_maps.append({"wall": wall.astype(bf16)})
    return in_maps


def _build_program(schedule):
    import concourse.bass as bass  # noqa: F401
    import concourse.bacc as bacc
    import concourse.tile as tile
    import concourse.mybir as mybir

    frags = schedule["frags"]
    groups = schedule["groups"]
    chains = schedule["chains"]
    wall_w = schedule["wall_w"]
    dma_plan = schedule["dma_plan"]
    n_batches = schedule["n_batches"]
    tot = schedule["tot"]
    n_frags = len(frags)
    f32 = mybir.dt.float32
    bf16 = mybir.dt.bfloat16

    nc = bacc.Bacc("TRN2", target_bir_lowering=False, debug=False,
                   num_devices=N_CORES)
    wall_d = nc.dram_tensor("wall", [128, wall_w], bf16, kind="ExternalInput")
    val_d = nc.dram_tensor("val", [128, n_frags], f32, kind="ExternalOutput")

    groups_by_batch = [[] for _ in range(n_batches)]
    for g in groups:
        groups_by_batch[g["g0"] // BATCH_COLS].append(g)
    chains_by_batch = [[] for _ in range(n_batches)]
    for ch in chains:
        chains_by_batch[ch["batch"]].append(ch)

    with tile.TileContext(nc) as tc:
        with tc.tile_pool(name="inp", bufs=1) as inp, \
             tc.tile_pool(name="stg", bufs=5) as stg, \
             tc.tile_pool(name="vp", bufs=1) as vp, \
             tc.tile_pool(name="ps", bufs=2, space="PSUM") as ps:
            wall_t = inp.tile([128, wall_w], bf16, name="wall_sb")
            val_t = vp.tile([128, n_frags], f32, name="val_sb")

            # exp-table warmup: tiny activation with no deps loads the Exp
            # table (~1.3 us) while the first input DMA is in flight
            warm = vp.tile([128, 4], f32, name="warm")
            nc.scalar.activation(warm, warm,
                                 mybir.ActivationFunctionType.Exp)

            # progressive input DMAs (one tight rectangle per batch span)
            for (rows, a, z) in dma_plan:
                nc.sync.dma_start(out=wall_t[0:rows, a:z],
                                  in_=wall_d.ap()[0:rows, a:z])

            batch_order = schedule["batch_order"]
            slot_lo = [min(ch["slot0"] for ch in chains_by_batch[b])
                       for b in range(n_batches)]
            slot_hi = [max(ch["slot0"] + ch["n"] for ch in chains_by_batch[b])
                       for b in range(n_batches)]
            mid_span = None
            for bi, b in enumerate(batch_order):
                used = min(tot - b * BATCH_COLS, BATCH_COLS)
                pt = ps.tile([128, BATCH_COLS], f32, name=f"pt{b}", tag="pt")
                for g in groups_by_batch[b]:
                    lo = g["g0"] - b * BATCH_COLS
                    nc.tensor.matmul(
                        pt[:, lo:lo + g["glen"]],
                        wall_t[0:g["rows"], g["lhs_ofs"]:g["lhs_ofs"] + 128],
                        wall_t[0:g["rows"], g["rhs_ofs"]:g["rhs_ofs"] + g["glen"]],
                        start=True, stop=True)
                st = stg.tile([128, BATCH_COLS], f32, name=f"st{b}", tag="st")
                if b == schedule["accum_batch"]:
                    # fused exp+reduce per item on ACT (tail batch)
                    for ch in chains_by_batch[b]:
                        for i in range(ch["n"]):
                            c0 = ch["ofs"] + i * ch["flen"]
                            nc.scalar.activation(
                                st[:, c0:c0 + ch["flen"]],
                                pt[:, c0:c0 + ch["flen"]],
                                mybir.ActivationFunctionType.Exp,
                                accum_out=val_t[:, ch["slot0"] + i:
                                                ch["slot0"] + i + 1])
                    continue
                nc.scalar.activation(st[:, :used], pt[:, :used],
                                     mybir.ActivationFunctionType.Exp)
                # Pool folds first, then straight DVE reduces, then the DVE
                # finishes of folded sub-chains (avoids DVE head-of-line
                # blocking on Pool).  The last n_p items of a chain are the
                # pool-folded sub-chain.
                def sub3(ch, i0, cnt, width):
                    ofs = ch["ofs"] + i0 * ch["flen"]
                    span = st[:, ofs:ofs + cnt * ch["flen"]]
                    if cnt == 1:
                        return span[:, 0:width]
                    s3 = span.rearrange("p (n l) -> p n l", l=ch["flen"])
                    return s3 if width == ch["flen"] else s3[:, :, 0:width]

                for ch in chains_by_batch[b]:
                    n_p = ch["n_p"]
                    if n_p:
                        s3 = sub3(ch, ch["n"] - n_p, n_p, ch["flen"])
                        full = ch["flen"]
                        for (L0, h, k) in ch["folds"]:
                            if n_p == 1:
                                nc.gpsimd.tensor_tensor(
                                    s3[:, 0:k], s3[:, 0:k],
                                    s3[:, h:h + k], mybir.AluOpType.add)
                            else:
                                nc.gpsimd.tensor_tensor(
                                    s3[:, :, 0:k], s3[:, :, 0:k],
                                    s3[:, :, h:h + k], mybir.AluOpType.add)
                for ch in chains_by_batch[b]:
                    n_v = ch["n"] - ch["n_p"]
                    if not n_v:
                        continue
                    nc.vector.tensor_reduce(
                        val_t[:, ch["slot0"]:ch["slot0"] + n_v],
                        sub3(ch, 0, n_v, ch["flen"]),
                        axis=mybir.AxisListType.X, op=mybir.AluOpType.add)
                for ch in chains_by_batch[b]:
                    n_p = ch["n_p"]
                    if not n_p:
                        continue
                    nc.vector.tensor_reduce(
                        val_t[:, ch["slot0"] + ch["n"] - n_p:
                              ch["slot0"] + ch["n"]],
                        sub3(ch, ch["n"] - n_p, n_p, ch["l_fin"]),
                        axis=mybir.AxisListType.X, op=mybir.AluOpType.add)
                if n_batches >= 5 and bi == n_batches - 5:
                    hi = max(slot_hi[x] for x in batch_order[:bi + 1])
                    mid_span = hi
                    nc.sync.dma_start(out=val_d.ap()[:, :hi],
                                      in_=val_t[:, :hi])
            if mid_span is None:
                nc.sync.dma_start(out=val_d.ap(), in_=val_t)
            else:
                nc.sync.dma_start(out=val_d.ap()[:, mid_span:],
                                  in_=val_t[:, mid_span:])

    nc.compile()
    return nc


def _assemble(schedule, per_core, results):
    occ = np.zeros((RES, RES, RES), np.float32)
    frags = schedule["frags"]
    acc = {}
    for c in range(N_CORES):
        val = results[c]["val"]
        for f in frags:
            ch = per_core[c][f["rank"]]
            if ch is None:
                continue
            _, bi, bj, m, _ = ch
            key = (bi, bj, m)
            v = val[:, f["slot"]]
            if key in acc:
                acc[key] = acc[key] + v
            else:
                acc[key] = v.copy()
    for (bi, bj, m), v in acc.items():
        v = v.reshape(2, 4, 4, 4)
        occ[bi * 4:bi * 4 + 4, bj * 4:bj * 4 + 4, 8 * m:8 * m + 4] = v[0]
        occ[bi * 4:bi * 4 + 4, bj * 4:bj * 4 + 4, 8 * m + 4:8 * m + 8] = v[1]
    return occ


def kernel(_xyz, _scaling, _rotation, _opacity, resolution, num_blocks):
    assert int(resolution) == RES and int(num_blocks) == NB, \
        f"kernel hardcoded for resolution=64 num_blocks=16, got {resolution}/{num_blocks}"
    try:
        import concourse.bass_utils as bass_utils  # noqa: F401
    except ImportError:
        import sys
        sys.path.insert(0, "/opt/trn_rl_repo")
        import concourse.bass_utils as bass_utils

    _xyz = np.asarray(_xyz, np.float32)
    _scaling = np.asarray(_scaling, np.float32)
    _rotation = np.asarray(_rotation, np.float32)
    _opacity = np.asarray(_opacity, np.float32)

    xyzs, opa, logopa, inv = _host_prep(_xyz, _scaling, _rotation, _opacity)
    schedule, per_core = _build_workload(xyzs, opa, logopa, inv)
    in_maps = _build_inputs(schedule, per_core, xyzs, logopa, inv)

    key = (schedule["m_items"], tuple(schedule["l_sched"]))
    if key not in _CACHE:
        _CACHE.clear()
        _CACHE[key] = _build_program(schedule)
    nc = _CACHE[key]

    # the axon tunnel occasionally reports a transient
    # NRT_EXEC_UNIT_UNRECOVERABLE; it clears on retry
    import time
    last_err = None
    for attempt in range(4):
        try:
            res = bass_utils.run_bass_kernel_spmd(
                nc, in_maps, core_ids=list(range(N_CORES)))
            return _assemble(schedule, per_core, res.results)
        except Exception as e:  # noqa: BLE001
            last_err = e
            if "UNRECOVERABLE" not in str(e) and "UNAVAILABLE" not in str(e):
                raise
            time.sleep(10 * (attempt + 1))
    raise last_err
